# revision 49
# baseline (speedup 1.0000x reference)
"""Trainium2 Bass kernel for nn_AttentionModeEncoder (B=4, S=2048, HID=1024, 16 heads x 64).

Sharding: 8 cores = 4 batches x 2 head-groups (8 heads / 512 features per core).

Key design points:
- Host pre-transposes + pre-casts operands to bf16 (x^T for the Q side, a
  mask-compacted x^T for the K/V side, W^T for all four weights).  No PE
  transposes; every matmul runs at 1 cycle/row (fp32 would be 4).
- Mask folding: the encoder mask only zeroes keys, so the host compacts K/V
  rows to the unmasked set (<=1044 of 2048, padded to SKV=1152 = the minimal
  9 k-tiles).  Scores, exp and AV shrink 9/16 vs full; padded rows
  contribute exactly 0 because their V rows AND the softmax-denominator
  ones-column are zeroed, so exp needs no mask bias.
- All per-partition constant tiles are pre-arranged [128, n] on the host so
  every DMA is contiguous (no 4-byte gather descriptors), and DMA triggers
  are emitted in need-order on the in-order queue.
- Head duplication (dup-half score packing) is done with partition-shifted
  DVE copies instead of SBUF-SBUF DMAs, keeping the DMA queue free.
- Phase B is ScalarE(exp)-bound, so the PE is kept warm (HAM clock gate!)
  by giving every attention unit a dense matmul burst: scores for unit u,
  AV matmuls for unit u-1 (software-pipelined; PTt fully ready), plus
  filler: Q^T projection chunks (units 0-5), scratch matmuls (units 6-8),
  and the first half of the out-projection (units 9-15, legal because
  units are ordered qc-major: all heads' q0:1024 attention output is done
  after unit 8).

Per core (batch b, head-group g):
  Phase A: V = x_kv @ WvT t-major (lands directly in the AV layout, ones
    column = padmask, bias via a K=1 rank-1 matmul), K^T j-major + Q^T jt=0.
  Phase B per unit (head, 1024-q chunk), qc-major order: S^T[k,q] =
    K^T.T @ Q^T with two k-tiles row-packed into the two PE partition
    halves (concurrent MMs), plain Exp on ScalarE (bf16 out), AV with the
    masked-ones row giving denominators, PE broadcast + fast reciprocal +
    DVE multiply for the normalize.
  Phase C: y^T = Wo^T.T @ attn^T (bf16, fp32 accum + bias) streamed out;
    first half runs as Phase-B filler, second half as the tail.
Host sums the two partials per batch and transposes.
"""

import os
import sys
import numpy as np
from contextlib import ExitStack

for _p in ("/opt/trn_rl_repo", "/root/.axon_site/_ro/trn_rl_repo"):
    if os.path.isdir(_p) and _p not in sys.path:
        sys.path.insert(0, _p)

import ml_dtypes
import concourse.bass as bass
import concourse.bacc as bacc
import concourse.mybir as mybir
import concourse.tile as tile

B, S, HID = 4, 2048, 1024
JC = 512                 # features per core (8 heads)
SKV = 1152               # padded compacted key/value length (9 k-tiles)
NKT = SKV // 128         # 9 k-tiles
NCORES = 8
FP = mybir.dt.float32
BF = mybir.dt.bfloat16
MULT = mybir.AluOpType.mult
EXP = mybir.ActivationFunctionType.Exp
BF_NP = ml_dtypes.bfloat16

TRACE = False
LAST_RESULTS = {}

# K/V t-chunks for the j-major K^T projection (SKV = 512 + 512 + 128)
KV_CHUNKS = [(0, 512), (512, 512), (1024, 128)]
# k-tile groups per attention unit.  Scores run the single k-tile FIRST so
# the next unit's first exp is ready after only two matmuls; AV consumes
# PTt (fully ready, one unit behind) in plain order so late V filler
# chunks (kt 7-8, units 0-1) land before their AV group.
SCORE_GROUPS = [(8,), (0, 1), (2, 3), (4, 5), (6, 7)]
AV_GROUPS = [(0, 1), (2, 3), (4, 5), (6, 7), (8,)]


def build_nc():
    nc = bacc.Bacc()
    xq = nc.declare_dram_parameter("xq", [HID, S], BF, isOutput=False)
    xkv = nc.declare_dram_parameter("xkv", [HID, SKV], BF, isOutput=False)
    pmrow = nc.declare_dram_parameter("pmrow", [1, SKV], BF, isOutput=False)
    pmcol = nc.declare_dram_parameter("pmcol", [128, NKT], FP, isOutput=False)
    wq = nc.declare_dram_parameter("wq", [HID, JC], BF, isOutput=False)
    bq = nc.declare_dram_parameter("bq", [128, 4], FP, isOutput=False)
    wk = nc.declare_dram_parameter("wk", [HID, JC], BF, isOutput=False)
    bk = nc.declare_dram_parameter("bk", [128, 4], FP, isOutput=False)
    wv = nc.declare_dram_parameter("wv", [HID, JC], BF, isOutput=False)
    bvrow = nc.declare_dram_parameter("bvrow", [1, JC], BF, isOutput=False)
    wo = nc.declare_dram_parameter("wo", [JC, HID], BF, isOutput=False)
    bo = nc.declare_dram_parameter("bo", [128, 8], FP, isOutput=False)
    y = nc.declare_dram_parameter("y", [HID, S], FP, isOutput=True)

    with tile.TileContext(nc) as tc, ExitStack() as ctx:
        const = ctx.enter_context(tc.tile_pool(name="const", bufs=1))
        mid = ctx.enter_context(tc.tile_pool(name="mid", bufs=1))
        qpool = ctx.enter_context(tc.tile_pool(name="qpool", bufs=1))
        kvpool = ctx.enter_context(tc.tile_pool(name="kvpool", bufs=1))
        wkctx = ExitStack()           # closed after Phase A
        wkpool = wkctx.enter_context(tc.tile_pool(name="wkpool", bufs=1))

        # ---- bulk loads, in need-order (in-order DMA queue) ----
        pmr = const.tile([1, SKV], BF)
        nc.sync.dma_start(out=pmr[:], in_=pmrow[:, :])
        bvr = const.tile([1, JC], BF)
        nc.sync.dma_start(out=bvr[:], in_=bvrow[:, :])

        wkT = wkpool.tile([128, 8, JC], BF)
        nc.sync.dma_start(out=wkT[:], in_=wk.rearrange("(it p) j -> p it j", p=128))
        xkvT = kvpool.tile([128, 8, SKV], BF)    # [i in tile, it, k]
        xkv_r = xkv.rearrange("(it p) t -> p it t", p=128)
        nc.sync.dma_start(out=xkvT[:, :, 0:640], in_=xkv_r[:, :, 0:640])
        wvT = kvpool.tile([128, 8, JC], BF)
        nc.sync.dma_start(out=wvT[:], in_=wv.rearrange("(it p) j -> p it j", p=128))
        nc.sync.dma_start(out=xkvT[:, :, 640:SKV], in_=xkv_r[:, :, 640:SKV])
        bkt = const.tile([128, 4], FP, tag="bk")
        nc.sync.dma_start(out=bkt[:], in_=bk[:, :])
        pmc = const.tile([128, NKT], FP)
        nc.sync.dma_start(out=pmc[:], in_=pmcol[:, :])
        bqt = const.tile([128, 4], FP, tag="bq")
        nc.sync.dma_start(out=bqt[:], in_=bq[:, :])
        xqT = qpool.tile([128, 8, S], BF)
        nc.sync.dma_start(
            out=xqT[:], in_=xq.rearrange("(it p) t -> p it t", p=128)
        )
        wqT = qpool.tile([128, 8, JC], BF)
        nc.sync.dma_start(out=wqT[:], in_=wq.rearrange("(it p) j -> p it j", p=128))
        bot = const.tile([128, 8], FP)
        nc.sync.dma_start(out=bot[:], in_=bo[:, :])
        woTs = mid.tile([128, 4, HID], BF)       # [c-part, ct, o]
        nc.sync.dma_start(out=woTs[:], in_=wo.rearrange("(ct p) o -> p ct o", p=128))

        ones1 = const.tile([1, 64], BF)
        nc.vector.memset(ones1[:], 1.0)
        ones8 = const.tile([128, 8, 1], BF)
        nc.vector.memset(ones8[:], 1.0)

        # persistent tensors.  QTd/KTd hold each head's 64 feature rows
        # DUPLICATED into both partition halves so k-tile pairs can be
        # row-packed into both halves of the PE array concurrently.
        KTd = mid.tile([128, 8, SKV], BF)        # [dup-half x d, head, k]
        QTd = mid.tile([128, 8, S], BF)
        vaug = mid.tile([128, NKT, 8, 65], BF)   # V aug: [k, kt, head, d|pad-ones]
        outT = mid.tile([128, 4, S], BF)         # attention out^T (c-major)

        def proj_copy(dst, jt, ps, bias, tslice):
            """psum [j,t] -> dst head tiles, native halves (DVE + bias)."""
            for hh in range(2):
                p0 = hh * 64
                nc.vector.tensor_scalar_add(
                    dst[p0:p0 + 64, jt * 2 + hh, tslice],
                    ps[p0:p0 + 64, 0:tslice.stop - tslice.start],
                    bias[p0:p0 + 64, jt:jt + 1],
                )

        def dup_heads(dst, jt, c0=0, c1=None):
            """Duplicate each head's 64 rows (cols c0:c1) into the opposite
            partition half (SBUF-SBUF DMA; the load queue is drained by
            now).  Q duplication is split into q-halves so the half a
            consumer unit needs is ready a unit earlier."""
            if c1 is None:
                c1 = dst.shape[-1]
            for hh in range(2):
                h = jt * 2 + hh
                srcp, dstp = hh * 64, 64 - hh * 64
                nc.sync.dma_start(
                    out=dst[dstp:dstp + 64, h, c0:c1],
                    in_=dst[srcp:srcp + 64, h, c0:c1],
                )

        def q_mms(ps, jt, tq, its, start, stop):
            t0 = tq * 512
            for it in its:
                nc.tensor.matmul(
                    ps[:],
                    lhsT=wqT[:, it, jt * 128:(jt + 1) * 128],
                    rhs=xqT[:, it, t0:t0 + 512],
                    start=(start and it == its[0]), stop=(stop and it == its[-1]),
                    skip_group_check=True,
                )

        def project_q_chunk(qpsum_pool, jt, tq):
            ps = qpsum_pool.tile([128, JC], FP, tag="qps")
            q_mms(ps, jt, tq, list(range(8)), True, True)
            proj_copy(QTd, jt, ps, bqt, slice(tq * 512, tq * 512 + 512))
            if tq == 1:
                dup_heads(QTd, jt, 0, 1024)
            elif tq == 3:
                dup_heads(QTd, jt, 1024, 2048)

        def q_pieces(jt, tq):
            """A Q-projection chunk as two 4-matmul pieces (finer filler
            interleave so ScalarE's exp queue is never outrun)."""
            st = {}

            def p1():
                st["ps"] = qpsum.tile([128, JC], FP, tag="qps", name="qps")
                q_mms(st["ps"], jt, tq, [0, 1, 2, 3], True, False)

            def p2():
                ps = st["ps"]
                q_mms(ps, jt, tq, [4, 5, 6, 7], False, True)
                proj_copy(QTd, jt, ps, bqt, slice(tq * 512, tq * 512 + 512))
                if tq == 3:
                    dup_heads(QTd, jt)

            return [p1, p2]

        def v_mms(ps, kt, its):
            for it in its:
                nc.tensor.matmul(
                    ps[:],
                    lhsT=xkvT[:, it, kt * 128:(kt + 1) * 128],
                    rhs=wvT[:, it, :],
                    start=(it == 0), stop=False,
                    skip_group_check=True,
                )

        def v_finish(ps, kt):
            nc.tensor.matmul(
                ps[:],
                lhsT=pmr[:, kt * 128:(kt + 1) * 128],
                rhs=bvr[:],
                start=False, stop=True,
                skip_group_check=True,
            )
            nc.vector.tensor_copy(out=vaug[:, kt, :, 0:64], in_=ps[:])
            nc.vector.tensor_scalar_mul(
                vaug[:, kt, :, 64:65], ones8[:], pmc[:, kt:kt + 1]
            )

        def project_v(pool, kt):
            ps = pool.tile([128, JC], FP, tag="qps", name="vps")
            v_mms(ps, kt, list(range(8)))
            v_finish(ps, kt)

        def v_pieces(kt):
            st = {}

            def p1():
                st["ps"] = qpsum.tile([128, JC], FP, tag="qps", name="vps")
                v_mms(st["ps"], kt, [0, 1, 2, 3])

            def p2():
                v_mms(st["ps"], kt, [4, 5, 6, 7])
                v_finish(st["ps"], kt)

            return [p1, p2]

        # ---------------- Phase A: K^T, V(kt0-3), Q^T(jt0) -----------------
        with ExitStack() as actx:
            apsum = actx.enter_context(tc.tile_pool(name="apsum", bufs=4, space="PSUM"))

            def project_k_chunk(jt, t0, tl):
                ps = apsum.tile([128, JC], FP, tag="aps")
                for it in range(8):
                    nc.tensor.matmul(
                        ps[:, 0:tl],
                        lhsT=wkT[:, it, jt * 128:(jt + 1) * 128],
                        rhs=xkvT[:, it, t0:t0 + tl],
                        start=(it == 0), stop=(it == 7),
                    )
                proj_copy(KTd, jt, ps, bkt, slice(t0, t0 + tl))

            # K0 chunk 1 and V kt0-3 only need xkv cols 0:640 (first piece)
            project_k_chunk(0, 0, 512)
            for kt in range(4):
                project_v(apsum, kt)
            for t0, tl in KV_CHUNKS[1:]:
                project_k_chunk(0, t0, tl)
            dup_heads(KTd, 0)
            for jt in range(1, 4):
                for t0, tl in KV_CHUNKS:
                    project_k_chunk(jt, t0, tl)
                dup_heads(KTd, jt)

            # Q^T jt=0 (pre-scaled by 0.125 on host); jt=1..3 and V kt4-8
            # run as Phase-B filler.
            for tq in range(4):
                project_q_chunk(apsum, 0, tq)

        wkctx.close()

        # ------------- Phase B: attention, AV pipelined one unit behind ----
        with ExitStack() as bctx:
            ptpool = bctx.enter_context(tc.tile_pool(name="ptpool", bufs=2))
            rpool = bctx.enter_context(tc.tile_pool(name="rpool", bufs=3))
            ypool = bctx.enter_context(tc.tile_pool(name="ypool", bufs=3))
            spool = bctx.enter_context(tc.tile_pool(name="spool", bufs=2, space="PSUM"))
            avpool = bctx.enter_context(
                tc.tile_pool(name="avpool", bufs=2, space="PSUM")
            )
            qpsum = bctx.enter_context(tc.tile_pool(name="qpsum", bufs=2, space="PSUM"))

            def emit_scores_group(h, qc, PTt, kts):
                """Score MMs + exp for k-tile group kts of unit (h, qc)."""
                q0 = qc * 1024
                sps = []
                for i, kt in enumerate(kts):
                    p0 = (kt % 2) * 64
                    sp = spool.tile([128, 1024], FP, tag="sp", name=f"sp{i}")
                    for qq in range(2):
                        qs = slice(q0 + qq * 512, q0 + (qq + 1) * 512)
                        nc.tensor.matmul(
                            sp[:, qq * 512:(qq + 1) * 512],
                            lhsT=KTd[p0:p0 + 64, h, kt * 128:(kt + 1) * 128],
                            rhs=QTd[p0:p0 + 64, h, qs],
                            start=True, stop=True,
                        )
                    sps.append(sp)
                for kt, sp in zip(kts, sps):
                    nc.scalar.activation(PTt[:, kt, :], sp[:], EXP)

            def emit_av_group(hp, qcp, PTp, avps, kts):
                for kt in kts:
                    for qq in range(2):
                        nc.tensor.matmul(
                            avps[qq][0:65, :],
                            lhsT=vaug[:, kt, hp, :],
                            rhs=PTp[:, kt, qq * 512:(qq + 1) * 512],
                            start=(kt == 0), stop=(kt == NKT - 1),
                            skip_group_check=True,
                        )

            def emit_norm(hp, qcp, avps):
                qp0 = qcp * 1024
                for qq in range(2):
                    avp = avps[qq]
                    s_sb = rpool.tile([1, 512], BF, tag="s_sb")
                    nc.vector.tensor_copy(out=s_sb[:], in_=avp[64:65, :])
                    sums_b = qpsum.tile([128, JC], FP, tag="qps", name="sums_b")
                    nc.tensor.matmul(
                        sums_b[0:64, :], lhsT=ones1[:], rhs=s_sb[:],
                        start=True, stop=True,
                    )
                    recb = rpool.tile([64, 512], FP, tag="recb")
                    nc.vector.reciprocal_approx_fast(recb[:], sums_b[0:64, :])
                    nc.vector.tensor_tensor(
                        outT[(hp % 2) * 64:(hp % 2) * 64 + 64, hp // 2,
                             qp0 + qq * 512:qp0 + (qq + 1) * 512],
                        avp[0:64, :], recb[:], MULT,
                    )

            def scratch_q_chunk():
                """Dummy Q-projection matmuls into scratch psum (PE warmth
                filler for units with no real filler work)."""
                ps = qpsum.tile([128, JC], FP, tag="qps", name="scratch")
                for it in range(8):
                    nc.tensor.matmul(
                        ps[:],
                        lhsT=wqT[:, it, 0:128],
                        rhs=xqT[:, it, 0:512],
                        start=(it == 0), stop=(it == 7),
                    )

            def c_chunk(ot, ts):
                """Out-projection for output rows [128*ot, +128), t slice
                [512*ts, +512)."""
                yps = qpsum.tile([128, JC], FP, tag="qps", name="cps")
                for ct in range(4):
                    nc.tensor.matmul(
                        yps[:],
                        lhsT=woTs[:, ct, ot * 128:(ot + 1) * 128],
                        rhs=outT[:, ct, ts * 512:(ts + 1) * 512],
                        start=(ct == 0), stop=(ct == 3),
                    )
                yt = ypool.tile([128, JC], FP, tag="yt")
                nc.vector.tensor_scalar_add(yt[:], yps[:], bot[:, ot:ot + 1])
                nc.sync.dma_start(
                    out=y[ot * 128:(ot + 1) * 128, ts * 512:(ts + 1) * 512],
                    in_=yt[:],
                )

            # qc-major unit order: all heads at q0:1024 first, then q1024:2048
            units = [(h, qc) for qc in range(2) for h in range(8)]
            # Filler schedule (emitted mid-unit, keeps the PE dense):
            #   units 0-5: two real Q chunks each (jt=1..3 x tq=0..3),
            #   units 6-8: two scratch chunks each,
            #   units 9-15: first-half out-projection chunks (outT q0:1024
            #   is complete once unit 8 has emitted norm for (h7, qc0)).
            filler = {u: [] for u in range(16)}
            filler[0] = [lambda kt=kt: project_v(qpsum, kt) for kt in (4, 5, 6)]
            filler[0] += [lambda tq=tq: project_q_chunk(qpsum, 1, tq)
                          for tq in (0, 1)]
            filler[1] = [lambda kt=kt: project_v(qpsum, kt) for kt in (7, 8)]
            filler[1] += [lambda tq=tq: project_q_chunk(qpsum, 1, tq)
                          for tq in (2, 3)]
            qjobs = [(jt, tq) for jt in range(2, 4) for tq in range(4)]
            for i, (jt, tq) in enumerate(qjobs):
                filler[2 + i // 2].append(
                    lambda jt=jt, tq=tq: project_q_chunk(qpsum, jt, tq))
            for u in range(6, 9):
                filler[u].append(scratch_q_chunk)
            cjobs0 = [(ot, ts) for ot in range(8) for ts in range(2)]
            for i, (ot, ts) in enumerate(cjobs0):
                filler[9 + i % 7].append(
                    lambda ot=ot, ts=ts: c_chunk(ot, ts)
                )

            pending = None           # (h, qc, PTt) of the unit awaiting AV
            for u, (h, qc) in enumerate(units):
                PTt = ptpool.tile([128, NKT, 1024], BF, tag="PT")
                avps = None
                # distribute filler jobs into the LAST len(jobs) group
                # slots (after that group's scores+AV), so ScalarE always
                # has queued exps covering each filler burst
                jobs = filler[u]
                ngroups = len(SCORE_GROUPS)
                for gi in range(ngroups):
                    emit_scores_group(h, qc, PTt, SCORE_GROUPS[gi])
                    if pending is not None:
                        if avps is None:
                            avps = [avpool.tile([128, JC], FP, tag="av",
                                                name=f"av{qq}")
                                    for qq in range(2)]
                        emit_av_group(pending[0], pending[1], pending[2],
                                      avps, AV_GROUPS[gi])
                    ji = gi - (ngroups - len(jobs))
                    if 0 <= ji < len(jobs):
                        jobs[ji]()
                if pending is not None:
                    emit_norm(pending[0], pending[1], avps)
                pending = (h, qc, PTt)
            # drain the last unit
            avps = [avpool.tile([128, JC], FP, tag="av", name=f"av{qq}")
                    for qq in range(2)]
            for kts in AV_GROUPS:
                emit_av_group(pending[0], pending[1], pending[2], avps, kts)
            emit_norm(pending[0], pending[1], avps)

        # ------------- Phase C tail: second-half out-projection ------------
        # Waves of 6 chunks: the ct0-2 partial accumulations depend only on
        # early norms, so they run while the drain normalize (which gates
        # every ct3 matmul) is still finishing on the Vector engine.
        with ExitStack() as cctx:
            cpsum = cctx.enter_context(tc.tile_pool(name="cpsum", bufs=6, space="PSUM"))
            cypool = cctx.enter_context(tc.tile_pool(name="cypool", bufs=6))
            chunks = [(ot, ts) for ot in range(8) for ts in range(2, 4)]
            for w0 in range(0, len(chunks), 6):
                wave = chunks[w0:w0 + 6]
                tiles = []
                for ot, ts in wave:
                    yps = cpsum.tile([128, JC], FP, tag="cps")
                    for ct in range(3):
                        nc.tensor.matmul(
                            yps[:],
                            lhsT=woTs[:, ct, ot * 128:(ot + 1) * 128],
                            rhs=outT[:, ct, ts * 512:(ts + 1) * 512],
                            start=(ct == 0), stop=False,
                            skip_group_check=True,
                        )
                    tiles.append(yps)
                for (ot, ts), yps in zip(wave, tiles):
                    nc.tensor.matmul(
                        yps[:],
                        lhsT=woTs[:, 3, ot * 128:(ot + 1) * 128],
                        rhs=outT[:, 3, ts * 512:(ts + 1) * 512],
                        start=False, stop=True,
                        skip_group_check=True,
                    )
                    yt = cypool.tile([128, JC], FP, tag="cyt")
                    nc.vector.tensor_scalar_add(yt[:], yps[:], bot[:, ot:ot + 1])
                    nc.sync.dma_start(
                        out=y[ot * 128:(ot + 1) * 128,
                              ts * 512:(ts + 1) * 512],
                        in_=yt[:],
                    )
    return nc


_NC = None


def _get_nc():
    global _NC
    if _NC is None:
        _NC = build_nc()
        _NC.finalize()   # run Bacc passes (reg alloc, wait splitting)
    return _NC


def make_in_maps(x, mask, Wq, bq, Wk, bk, Wv, bv, Wo, bo):
    f32 = lambda a: np.ascontiguousarray(np.asarray(a, dtype=np.float32))
    bf = lambda a: np.ascontiguousarray(
        np.asarray(a, dtype=np.float32).astype(BF_NP)
    )
    p128 = lambda a, n: np.ascontiguousarray(
        np.asarray(a, dtype=np.float32).reshape(n, 128).T
    )
    x = np.asarray(x, dtype=np.float32)
    mask = np.asarray(mask)

    per_batch = []
    for b in range(B):
        idx = np.nonzero(mask[b] != 0)[0]
        n = len(idx)
        assert n <= SKV, f"batch {b}: {n} unmasked keys > SKV={SKV}"
        xkv = np.zeros((SKV, HID), np.float32)
        xkv[:n] = x[b][idx]
        pm = np.zeros(SKV, np.float32)
        pm[:n] = 1.0
        per_batch.append({
            "xq": bf(x[b].T),
            "xkv": bf(xkv.T),
            "pmrow": bf(pm.reshape(1, SKV)),
            "pmcol": p128(pm, NKT),
        })

    per_group = []
    for g in range(2):
        sl = slice(g * JC, (g + 1) * JC)
        per_group.append({
            "wq": bf(np.asarray(Wq)[sl].T * 0.125),
            "bq": p128(np.asarray(bq)[sl] * 0.125, 4),
            "wk": bf(np.asarray(Wk)[sl].T),
            "bk": p128(np.asarray(bk)[sl], 4),
            "wv": bf(np.asarray(Wv)[sl].T),
            "bvrow": bf(np.asarray(bv)[sl].reshape(1, JC)),
            "wo": bf(np.asarray(Wo)[:, sl].T),
            "bo": p128(bo, 8) if g == 0 else np.zeros((128, 8), np.float32),
        })

    in_maps = []
    for c in range(NCORES):
        b, g = c // 2, c % 2
        m = {}
        m.update(per_batch[b])
        m.update(per_group[g])
        in_maps.append(m)
    return in_maps


def kernel(x, mask, Wq, bq, Wk, bk, Wv, bv, Wo, bo):
    from concourse.bass_utils import run_bass_kernel_spmd

    nc = _get_nc()
    in_maps = make_in_maps(x, mask, Wq, bq, Wk, bk, Wv, bv, Wo, bo)
    kw = {}
    if TRACE:
        os.makedirs("/root/problem/trace_out", exist_ok=True)
        kw = dict(tmpdir="/root/problem/trace_out")
    r = run_bass_kernel_spmd(nc, in_maps, list(range(NCORES)), trace=TRACE, **kw)
    LAST_RESULTS["exec_time_ns"] = r.exec_time_ns
    LAST_RESULTS["mean_exec_time_ns"] = r.mean_exec_time_ns
    y = np.empty((B, S, HID), np.float32)
    for b in range(B):
        y[b] = (np.asarray(r.results[2 * b]["y"], np.float32)
                + np.asarray(r.results[2 * b + 1]["y"], np.float32)).T
    return y


# revision 52
# speedup vs baseline: 1.0118x; 1.0118x over previous
"""Trainium2 Bass kernel for nn_AttentionModeEncoder (B=4, S=2048, HID=1024, 16 heads x 64).

Sharding: 8 cores = 4 batches x 2 head-groups (8 heads / 512 features per core).

Key design points:
- Host pre-transposes + pre-casts operands to bf16 (x^T for the Q side, a
  mask-compacted x^T for the K/V side, W^T for all four weights).  No PE
  transposes; every matmul runs at 1 cycle/row (fp32 would be 4).
- Mask folding: the encoder mask only zeroes keys, so the host compacts K/V
  rows to the unmasked set (<=1044 of 2048, padded to SKV=1152 = the minimal
  9 k-tiles).  Scores, exp and AV shrink 9/16 vs full; padded rows
  contribute exactly 0 because their V rows AND the softmax-denominator
  ones-column are zeroed, so exp needs no mask bias.
- All per-partition constant tiles are pre-arranged [128, n] on the host so
  every DMA is contiguous (no 4-byte gather descriptors), and DMA triggers
  are emitted in need-order on the in-order queue.
- Head duplication (dup-half score packing) is done with partition-shifted
  DVE copies instead of SBUF-SBUF DMAs, keeping the DMA queue free.
- Phase B is ScalarE(exp)-bound, so the PE is kept warm (HAM clock gate!)
  by giving every attention unit a dense matmul burst: scores for unit u,
  AV matmuls for unit u-1 (software-pipelined; PTt fully ready), plus
  filler: Q^T projection chunks (units 0-5), scratch matmuls (units 6-8),
  and the first half of the out-projection (units 9-15, legal because
  units are ordered qc-major: all heads' q0:1024 attention output is done
  after unit 8).

Per core (batch b, head-group g):
  Phase A: V = x_kv @ WvT t-major (lands directly in the AV layout, ones
    column = padmask, bias via a K=1 rank-1 matmul), K^T j-major + Q^T jt=0.
  Phase B per unit (head, 1024-q chunk), qc-major order: S^T[k,q] =
    K^T.T @ Q^T with two k-tiles row-packed into the two PE partition
    halves (concurrent MMs), plain Exp on ScalarE (bf16 out), AV with the
    masked-ones row giving denominators, PE broadcast + fast reciprocal +
    DVE multiply for the normalize.
  Phase C: y^T = Wo^T.T @ attn^T (bf16, fp32 accum + bias) streamed out;
    first half runs as Phase-B filler, second half as the tail.
Host sums the two partials per batch and transposes.
"""

import os
import sys
import numpy as np
from contextlib import ExitStack

for _p in ("/opt/trn_rl_repo", "/root/.axon_site/_ro/trn_rl_repo"):
    if os.path.isdir(_p) and _p not in sys.path:
        sys.path.insert(0, _p)

import ml_dtypes
import concourse.bass as bass
import concourse.bacc as bacc
import concourse.mybir as mybir
import concourse.tile as tile

B, S, HID = 4, 2048, 1024
JC = 512                 # features per core (8 heads)
SKV = 1152               # padded compacted key/value length (9 k-tiles)
NKT = SKV // 128         # 9 k-tiles
NCORES = 8
FP = mybir.dt.float32
BF = mybir.dt.bfloat16
MULT = mybir.AluOpType.mult
EXP = mybir.ActivationFunctionType.Exp
BF_NP = ml_dtypes.bfloat16

TRACE = False
LAST_RESULTS = {}

# K/V t-chunks for the j-major K^T projection (SKV = 512 + 512 + 128)
KV_CHUNKS = [(0, 512), (512, 512), (1024, 128)]
# k-tile groups per attention unit.  Scores run the single k-tile FIRST so
# the next unit's first exp is ready after only two matmuls; AV consumes
# PTt (fully ready, one unit behind) in plain order so late V filler
# chunks (kt 7-8, units 0-1) land before their AV group.
SCORE_GROUPS = [(8,), (0, 1), (2, 3), (4, 5), (6, 7)]
AV_GROUPS = [(0, 1), (2, 3), (4, 5), (6, 7), (8,)]


def build_nc():
    nc = bacc.Bacc()
    xq = nc.declare_dram_parameter("xq", [HID, S], BF, isOutput=False)
    xkv = nc.declare_dram_parameter("xkv", [HID, SKV], BF, isOutput=False)
    pmrow = nc.declare_dram_parameter("pmrow", [1, SKV], BF, isOutput=False)
    pmcol = nc.declare_dram_parameter("pmcol", [128, NKT], FP, isOutput=False)
    wq = nc.declare_dram_parameter("wq", [HID, JC], BF, isOutput=False)
    bq = nc.declare_dram_parameter("bq", [128, 4], FP, isOutput=False)
    wk = nc.declare_dram_parameter("wk", [HID, JC], BF, isOutput=False)
    bk = nc.declare_dram_parameter("bk", [128, 4], FP, isOutput=False)
    wv = nc.declare_dram_parameter("wv", [HID, JC], BF, isOutput=False)
    bvrow = nc.declare_dram_parameter("bvrow", [1, JC], BF, isOutput=False)
    wo = nc.declare_dram_parameter("wo", [JC, HID], BF, isOutput=False)
    bo = nc.declare_dram_parameter("bo", [128, 8], FP, isOutput=False)
    y = nc.declare_dram_parameter("y", [HID, S], FP, isOutput=True)

    with tile.TileContext(nc) as tc, ExitStack() as ctx:
        const = ctx.enter_context(tc.tile_pool(name="const", bufs=1))
        mid = ctx.enter_context(tc.tile_pool(name="mid", bufs=1))
        qpool = ctx.enter_context(tc.tile_pool(name="qpool", bufs=1))
        kvpool = ctx.enter_context(tc.tile_pool(name="kvpool", bufs=1))
        wkctx = ExitStack()           # closed after Phase A
        wkpool = wkctx.enter_context(tc.tile_pool(name="wkpool", bufs=1))

        # ---- bulk loads, in need-order (in-order DMA queue) ----
        pmr = const.tile([1, SKV], BF)
        nc.sync.dma_start(out=pmr[:], in_=pmrow[:, :])
        bvr = const.tile([1, JC], BF)
        nc.sync.dma_start(out=bvr[:], in_=bvrow[:, :])

        wkT = wkpool.tile([128, 8, JC], BF)
        nc.sync.dma_start(out=wkT[:], in_=wk.rearrange("(it p) j -> p it j", p=128))
        xkvT = kvpool.tile([128, 8, SKV], BF)    # [i in tile, it, k]
        xkv_r = xkv.rearrange("(it p) t -> p it t", p=128)
        nc.sync.dma_start(out=xkvT[:, :, 0:640], in_=xkv_r[:, :, 0:640])
        wvT = kvpool.tile([128, 8, JC], BF)
        nc.sync.dma_start(out=wvT[:], in_=wv.rearrange("(it p) j -> p it j", p=128))
        nc.sync.dma_start(out=xkvT[:, :, 640:SKV], in_=xkv_r[:, :, 640:SKV])
        bkt = const.tile([128, 4], FP, tag="bk")
        nc.sync.dma_start(out=bkt[:], in_=bk[:, :])
        pmc = const.tile([128, NKT], FP)
        nc.sync.dma_start(out=pmc[:], in_=pmcol[:, :])
        bqt = const.tile([128, 4], FP, tag="bq")
        nc.sync.dma_start(out=bqt[:], in_=bq[:, :])
        xqT = qpool.tile([128, 8, S], BF)
        nc.sync.dma_start(
            out=xqT[:], in_=xq.rearrange("(it p) t -> p it t", p=128)
        )
        wqT = qpool.tile([128, 8, JC], BF)
        nc.sync.dma_start(out=wqT[:], in_=wq.rearrange("(it p) j -> p it j", p=128))
        bot = const.tile([128, 8], FP)
        nc.sync.dma_start(out=bot[:], in_=bo[:, :])
        woTs = mid.tile([128, 4, HID], BF)       # [c-part, ct, o]
        nc.sync.dma_start(out=woTs[:], in_=wo.rearrange("(ct p) o -> p ct o", p=128))

        ones1 = const.tile([1, 64], BF)
        nc.vector.memset(ones1[:], 1.0)
        ones8 = const.tile([128, 8, 1], BF)
        nc.vector.memset(ones8[:], 1.0)

        # persistent tensors.  QTd/KTd hold each head's 64 feature rows
        # DUPLICATED into both partition halves so k-tile pairs can be
        # row-packed into both halves of the PE array concurrently.
        KTd = mid.tile([128, 8, SKV], BF)        # [dup-half x d, head, k]
        QTd = mid.tile([128, 8, S], BF)
        vaug = mid.tile([128, NKT, 8, 65], BF)   # V aug: [k, kt, head, d|pad-ones]
        outT = mid.tile([128, 4, S], BF)         # attention out^T (c-major)

        def proj_copy(dst, jt, ps, bias, tslice):
            """psum [j,t] -> dst head tiles, native halves (DVE + bias)."""
            for hh in range(2):
                p0 = hh * 64
                nc.vector.tensor_scalar_add(
                    dst[p0:p0 + 64, jt * 2 + hh, tslice],
                    ps[p0:p0 + 64, 0:tslice.stop - tslice.start],
                    bias[p0:p0 + 64, jt:jt + 1],
                )

        def dup_heads(dst, jt, c0=0, c1=None):
            """Duplicate each head's 64 rows (cols c0:c1) into the opposite
            partition half (SBUF-SBUF DMA; the load queue is drained by
            now).  Q duplication is split into q-halves so the half a
            consumer unit needs is ready a unit earlier."""
            if c1 is None:
                c1 = dst.shape[-1]
            for hh in range(2):
                h = jt * 2 + hh
                srcp, dstp = hh * 64, 64 - hh * 64
                nc.sync.dma_start(
                    out=dst[dstp:dstp + 64, h, c0:c1],
                    in_=dst[srcp:srcp + 64, h, c0:c1],
                )

        def q_mms(ps, jt, tq, its, start, stop):
            t0 = tq * 512
            for it in its:
                nc.tensor.matmul(
                    ps[:],
                    lhsT=wqT[:, it, jt * 128:(jt + 1) * 128],
                    rhs=xqT[:, it, t0:t0 + 512],
                    start=(start and it == its[0]), stop=(stop and it == its[-1]),
                    skip_group_check=True,
                )

        def project_q_chunk(qpsum_pool, jt, tq):
            ps = qpsum_pool.tile([128, JC], FP, tag="qps")
            q_mms(ps, jt, tq, list(range(8)), True, True)
            proj_copy(QTd, jt, ps, bqt, slice(tq * 512, tq * 512 + 512))
            if tq == 1:
                dup_heads(QTd, jt, 0, 1024)
            elif tq == 3:
                dup_heads(QTd, jt, 1024, 2048)

        def q_pieces(jt, tq):
            """A Q-projection chunk as two 4-matmul pieces (finer filler
            interleave so ScalarE's exp queue is never outrun)."""
            st = {}

            def p1():
                st["ps"] = qpsum.tile([128, JC], FP, tag="qps", name="qps")
                q_mms(st["ps"], jt, tq, [0, 1, 2, 3], True, False)

            def p2():
                ps = st["ps"]
                q_mms(ps, jt, tq, [4, 5, 6, 7], False, True)
                proj_copy(QTd, jt, ps, bqt, slice(tq * 512, tq * 512 + 512))
                if tq == 3:
                    dup_heads(QTd, jt)

            return [p1, p2]

        def v_mms(ps, kt, its):
            for it in its:
                nc.tensor.matmul(
                    ps[:],
                    lhsT=xkvT[:, it, kt * 128:(kt + 1) * 128],
                    rhs=wvT[:, it, :],
                    start=(it == 0), stop=False,
                    skip_group_check=True,
                )

        def v_finish(ps, kt):
            nc.tensor.matmul(
                ps[:],
                lhsT=pmr[:, kt * 128:(kt + 1) * 128],
                rhs=bvr[:],
                start=False, stop=True,
                skip_group_check=True,
            )
            nc.vector.tensor_copy(out=vaug[:, kt, :, 0:64], in_=ps[:])
            nc.vector.tensor_scalar_mul(
                vaug[:, kt, :, 64:65], ones8[:], pmc[:, kt:kt + 1]
            )

        def project_v(pool, kt):
            ps = pool.tile([128, JC], FP, tag="qps", name="vps")
            v_mms(ps, kt, list(range(8)))
            v_finish(ps, kt)

        def v_pieces(kt):
            st = {}

            def p1():
                st["ps"] = qpsum.tile([128, JC], FP, tag="qps", name="vps")
                v_mms(st["ps"], kt, [0, 1, 2, 3])

            def p2():
                v_mms(st["ps"], kt, [4, 5, 6, 7])
                v_finish(st["ps"], kt)

            return [p1, p2]

        # ---------------- Phase A: K^T, V(kt0-3), Q^T(jt0) -----------------
        with ExitStack() as actx:
            apsum = actx.enter_context(tc.tile_pool(name="apsum", bufs=4, space="PSUM"))

            def project_k_chunk(jt, t0, tl):
                ps = apsum.tile([128, JC], FP, tag="aps")
                for it in range(8):
                    nc.tensor.matmul(
                        ps[:, 0:tl],
                        lhsT=wkT[:, it, jt * 128:(jt + 1) * 128],
                        rhs=xkvT[:, it, t0:t0 + tl],
                        start=(it == 0), stop=(it == 7),
                    )
                proj_copy(KTd, jt, ps, bkt, slice(t0, t0 + tl))

            # K0 chunk 1 and V kt0-3 only need xkv cols 0:640 (first piece)
            project_k_chunk(0, 0, 512)
            for kt in range(4):
                project_v(apsum, kt)
            for t0, tl in KV_CHUNKS[1:]:
                project_k_chunk(0, t0, tl)
            dup_heads(KTd, 0)
            for jt in range(1, 4):
                for t0, tl in KV_CHUNKS:
                    project_k_chunk(jt, t0, tl)
                dup_heads(KTd, jt)

            # Q^T jt=0 (pre-scaled by 0.125 on host); jt=1..3 and V kt4-8
            # run as Phase-B filler.
            for tq in range(4):
                project_q_chunk(apsum, 0, tq)

        wkctx.close()

        # ------------- Phase B: attention, AV pipelined one unit behind ----
        with ExitStack() as bctx:
            ptpool = bctx.enter_context(tc.tile_pool(name="ptpool", bufs=2))
            rpool = bctx.enter_context(tc.tile_pool(name="rpool", bufs=3))
            ypool = bctx.enter_context(tc.tile_pool(name="ypool", bufs=3))
            spool = bctx.enter_context(tc.tile_pool(name="spool", bufs=2, space="PSUM"))
            avpool = bctx.enter_context(
                tc.tile_pool(name="avpool", bufs=2, space="PSUM")
            )
            qpsum = bctx.enter_context(tc.tile_pool(name="qpsum", bufs=2, space="PSUM"))

            def emit_scores_group(h, qc, PTt, kts):
                """Score MMs + exp for k-tile group kts of unit (h, qc)."""
                q0 = qc * 1024
                sps = []
                for i, kt in enumerate(kts):
                    p0 = (kt % 2) * 64
                    sp = spool.tile([128, 1024], FP, tag="sp", name=f"sp{i}")
                    for qq in range(2):
                        qs = slice(q0 + qq * 512, q0 + (qq + 1) * 512)
                        nc.tensor.matmul(
                            sp[:, qq * 512:(qq + 1) * 512],
                            lhsT=KTd[p0:p0 + 64, h, kt * 128:(kt + 1) * 128],
                            rhs=QTd[p0:p0 + 64, h, qs],
                            start=True, stop=True,
                        )
                    sps.append(sp)
                for kt, sp in zip(kts, sps):
                    nc.scalar.activation(PTt[:, kt, :], sp[:], EXP)

            def emit_av_group(hp, qcp, PTp, avps, kts):
                for kt in kts:
                    for qq in range(2):
                        nc.tensor.matmul(
                            avps[qq][0:65, :],
                            lhsT=vaug[:, kt, hp, :],
                            rhs=PTp[:, kt, qq * 512:(qq + 1) * 512],
                            start=(kt == 0), stop=(kt == NKT - 1),
                            skip_group_check=True,
                        )

            def emit_norm(hp, qcp, avps):
                qp0 = qcp * 1024
                for qq in range(2):
                    avp = avps[qq]
                    s_sb = rpool.tile([1, 512], BF, tag="s_sb")
                    nc.vector.tensor_copy(out=s_sb[:], in_=avp[64:65, :])
                    sums_b = qpsum.tile([128, JC], FP, tag="qps", name="sums_b")
                    nc.tensor.matmul(
                        sums_b[0:64, :], lhsT=ones1[:], rhs=s_sb[:],
                        start=True, stop=True,
                    )
                    recb = rpool.tile([64, 512], FP, tag="recb")
                    nc.vector.reciprocal_approx_fast(recb[:], sums_b[0:64, :])
                    nc.vector.tensor_tensor(
                        outT[(hp % 2) * 64:(hp % 2) * 64 + 64, hp // 2,
                             qp0 + qq * 512:qp0 + (qq + 1) * 512],
                        avp[0:64, :], recb[:], MULT,
                    )

            def scratch_q_chunk():
                """Dummy Q-projection matmuls into scratch psum (PE warmth
                filler for units with no real filler work)."""
                ps = qpsum.tile([128, JC], FP, tag="qps", name="scratch")
                for it in range(8):
                    nc.tensor.matmul(
                        ps[:],
                        lhsT=wqT[:, it, 0:128],
                        rhs=xqT[:, it, 0:512],
                        start=(it == 0), stop=(it == 7),
                    )

            def c_chunk(ot, ts):
                """Out-projection for output rows [128*ot, +128), t slice
                [512*ts, +512)."""
                yps = qpsum.tile([128, JC], FP, tag="qps", name="cps")
                for ct in range(4):
                    nc.tensor.matmul(
                        yps[:],
                        lhsT=woTs[:, ct, ot * 128:(ot + 1) * 128],
                        rhs=outT[:, ct, ts * 512:(ts + 1) * 512],
                        start=(ct == 0), stop=(ct == 3),
                    )
                yt = ypool.tile([128, JC], FP, tag="yt")
                nc.vector.tensor_scalar_add(yt[:], yps[:], bot[:, ot:ot + 1])
                nc.sync.dma_start(
                    out=y[ot * 128:(ot + 1) * 128, ts * 512:(ts + 1) * 512],
                    in_=yt[:],
                )

            # qc-major unit order: all heads at q0:1024 first, then q1024:2048
            units = [(h, qc) for qc in range(2) for h in range(8)]
            # Filler schedule (emitted mid-unit, keeps the PE dense):
            #   units 0-5: two real Q chunks each (jt=1..3 x tq=0..3),
            #   units 6-8: two scratch chunks each,
            #   units 9-15: first-half out-projection chunks (outT q0:1024
            #   is complete once unit 8 has emitted norm for (h7, qc0)).
            filler = {u: [] for u in range(16)}
            filler[0] = [lambda kt=kt: project_v(qpsum, kt) for kt in (4, 5, 6)]
            filler[0] += [lambda tq=tq: project_q_chunk(qpsum, 1, tq)
                          for tq in (0, 1)]
            filler[1] = [lambda kt=kt: project_v(qpsum, kt) for kt in (7, 8)]
            filler[1] += [lambda tq=tq: project_q_chunk(qpsum, 1, tq)
                          for tq in (2, 3)]
            qjobs = [(jt, tq) for jt in range(2, 4) for tq in range(4)]
            for i, (jt, tq) in enumerate(qjobs):
                filler[2 + i // 2].append(
                    lambda jt=jt, tq=tq: project_q_chunk(qpsum, jt, tq))
            for u in range(6, 9):
                filler[u].append(scratch_q_chunk)
            cjobs0 = [(ot, ts) for ot in range(8) for ts in range(2)]
            for i, (ot, ts) in enumerate(cjobs0):
                filler[9 + i % 7].append(
                    lambda ot=ot, ts=ts: c_chunk(ot, ts)
                )

            pending = None           # (h, qc, PTt) of the unit awaiting AV
            for u, (h, qc) in enumerate(units):
                PTt = ptpool.tile([128, NKT, 1024], BF, tag="PT")
                avps = None
                # distribute filler jobs into the LAST len(jobs) group
                # slots (after that group's scores+AV), so ScalarE always
                # has queued exps covering each filler burst
                jobs = filler[u]
                ngroups = len(SCORE_GROUPS)
                for gi in range(ngroups):
                    emit_scores_group(h, qc, PTt, SCORE_GROUPS[gi])
                    if pending is not None:
                        if avps is None:
                            avps = [avpool.tile([128, JC], FP, tag="av",
                                                name=f"av{qq}")
                                    for qq in range(2)]
                        emit_av_group(pending[0], pending[1], pending[2],
                                      avps, AV_GROUPS[gi])
                    ji = gi - (ngroups - len(jobs))
                    if 0 <= ji < len(jobs):
                        jobs[ji]()
                if pending is not None:
                    emit_norm(pending[0], pending[1], avps)
                pending = (h, qc, PTt)
            # drain the last unit
            avps = [avpool.tile([128, JC], FP, tag="av", name=f"av{qq}")
                    for qq in range(2)]
            for kts in AV_GROUPS:
                emit_av_group(pending[0], pending[1], pending[2], avps, kts)
            emit_norm(pending[0], pending[1], avps)

        # ------------- Phase C tail: second-half out-projection ------------
        with ExitStack() as cctx:
            cpsum = cctx.enter_context(tc.tile_pool(name="cpsum", bufs=6, space="PSUM"))
            cypool = cctx.enter_context(tc.tile_pool(name="cypool", bufs=6))
            for ot in range(8):
                for ts in range(2, 4):
                    yps = cpsum.tile([128, JC], FP, tag="cps")
                    for ct in range(4):
                        nc.tensor.matmul(
                            yps[:],
                            lhsT=woTs[:, ct, ot * 128:(ot + 1) * 128],
                            rhs=outT[:, ct, ts * 512:(ts + 1) * 512],
                            start=(ct == 0), stop=(ct == 3),
                        )
                    yt = cypool.tile([128, JC], FP, tag="cyt")
                    nc.vector.tensor_scalar_add(yt[:], yps[:], bot[:, ot:ot + 1])
                    nc.sync.dma_start(
                        out=y[ot * 128:(ot + 1) * 128, ts * 512:(ts + 1) * 512],
                        in_=yt[:],
                    )
    return nc


_NC = None


def _get_nc():
    global _NC
    if _NC is None:
        _NC = build_nc()
        _NC.finalize()   # run Bacc passes (reg alloc, wait splitting)
    return _NC


def make_in_maps(x, mask, Wq, bq, Wk, bk, Wv, bv, Wo, bo):
    f32 = lambda a: np.ascontiguousarray(np.asarray(a, dtype=np.float32))
    bf = lambda a: np.ascontiguousarray(
        np.asarray(a, dtype=np.float32).astype(BF_NP)
    )
    p128 = lambda a, n: np.ascontiguousarray(
        np.asarray(a, dtype=np.float32).reshape(n, 128).T
    )
    x = np.asarray(x, dtype=np.float32)
    mask = np.asarray(mask)

    per_batch = []
    for b in range(B):
        idx = np.nonzero(mask[b] != 0)[0]
        n = len(idx)
        assert n <= SKV, f"batch {b}: {n} unmasked keys > SKV={SKV}"
        xkv = np.zeros((SKV, HID), np.float32)
        xkv[:n] = x[b][idx]
        pm = np.zeros(SKV, np.float32)
        pm[:n] = 1.0
        per_batch.append({
            "xq": bf(x[b].T),
            "xkv": bf(xkv.T),
            "pmrow": bf(pm.reshape(1, SKV)),
            "pmcol": p128(pm, NKT),
        })

    per_group = []
    for g in range(2):
        sl = slice(g * JC, (g + 1) * JC)
        per_group.append({
            "wq": bf(np.asarray(Wq)[sl].T * 0.125),
            "bq": p128(np.asarray(bq)[sl] * 0.125, 4),
            "wk": bf(np.asarray(Wk)[sl].T),
            "bk": p128(np.asarray(bk)[sl], 4),
            "wv": bf(np.asarray(Wv)[sl].T),
            "bvrow": bf(np.asarray(bv)[sl].reshape(1, JC)),
            "wo": bf(np.asarray(Wo)[:, sl].T),
            "bo": p128(bo, 8) if g == 0 else np.zeros((128, 8), np.float32),
        })

    in_maps = []
    for c in range(NCORES):
        b, g = c // 2, c % 2
        m = {}
        m.update(per_batch[b])
        m.update(per_group[g])
        in_maps.append(m)
    return in_maps


def kernel(x, mask, Wq, bq, Wk, bk, Wv, bv, Wo, bo):
    from concourse.bass_utils import run_bass_kernel_spmd

    nc = _get_nc()
    in_maps = make_in_maps(x, mask, Wq, bq, Wk, bk, Wv, bv, Wo, bo)
    kw = {}
    if TRACE:
        os.makedirs("/root/problem/trace_out", exist_ok=True)
        kw = dict(tmpdir="/root/problem/trace_out")
    r = run_bass_kernel_spmd(nc, in_maps, list(range(NCORES)), trace=TRACE, **kw)
    LAST_RESULTS["exec_time_ns"] = r.exec_time_ns
    LAST_RESULTS["mean_exec_time_ns"] = r.mean_exec_time_ns
    y = np.empty((B, S, HID), np.float32)
    for b in range(B):
        y[b] = (np.asarray(r.results[2 * b]["y"], np.float32)
                + np.asarray(r.results[2 * b + 1]["y"], np.float32)).T
    return y
